# revision 1
# baseline (speedup 1.0000x reference)
"""Trainium2 Bass kernel for EntmaxBisectLoss (alpha=1.5) on [4096, 32000] f32.

Rows sharded across 8 NeuronCores (512 rows/core, 4 groups of 128
partition-rows). Per row the entmax threshold t* solves
    V(t) = sum_j relu(x_j - t)^2 = 4        (x-space; tau = t/2)
V is convex decreasing; a bracketed Newton/secant iteration converges in 4
evaluations from t0 = rowmax-1 within the bracket
[rowmax-2, rowmax-2*(1/d)^.5]. The final evaluation's V doubles as
W = sum relu^2; A = sum relu^3 and x_tgt = x[row, target] complete the loss:
    loss = (1 - A/W^1.5)/0.75 + A/W + t - x_tgt
(loss is stationary in t at t*, so threshold error is quadratically
suppressed).

Layout/engine plan per group:
  - stream fp32 in 2000-col chunks, fused fp16-convert + running row max on
    DVE (custom op, accum=max)  ->  fp16 tile [128, 32000] (two in flight)
  - R0/R2 on ACT: relu(bias=-t)->scratch (+S1 accum), square(scratch)->dump
    (+V accum); Newton step with exact derivative V' = -2*S1
  - R1 (secant) and A on DVE custom fused reduce ops with dump outputs
  - R3 (final eval, V reused as W) split by columns: head on DVE, tail on
    ACT, to balance engine load
  - x[row, target] via GPSIMD indirect_copy (16-wide group gather) + a tiny
    Idx-select
  - loss assembled once for all 4 groups ([128,4] vectors), partition-reduced
    by a ones-matmul; host sums the 8 per-core partials.
"""
import sys
sys.path.insert(0, "/opt/trn_rl_repo")

from contextlib import ExitStack
from operator import add as _add

import numpy as np

import concourse.bass as bass
import concourse.bacc as bacc
import concourse.tile as tile
from concourse import mybir
from concourse.bass_utils import run_bass_kernel_spmd
from concourse.dve_ops import (
    DveOp, OPS, CUSTOM_DVE_SPECS, _SUB_OPCODE_FOR_NAME,
    has_src1,
)
from concourse.dve_spec import (
    Spec, Src0, C0, C1, Idx, Zero, relu, sq, select, eq, lower, maxx,
)
from concourse.dve_uop import DveOpSpec

N_CORES = 8
N_ROWS = 4096
V_DIM = 32000
ROWS_PER_CORE = N_ROWS // N_CORES          # 512
P = 128
GROUPS = ROWS_PER_CORE // P                # 4
STAGE_CH = 2000                            # fp32 staging chunk cols
ACT_CH = 4000                              # ACT round chunk cols
DVE_CH = 2000                              # DVE custom-op chunk cols
SPL_DVE_CH = 12                            # split rounds: DVE_CH chunks on DVE
SPL_ACT_CH = 4000                         # split rounds: ACT-tail chunk size
DUMP_COLS = 500
HI_OFF = 2.0 * (1.0 / V_DIM) ** 0.5

F32 = mybir.dt.float32
F16 = mybir.dt.float16
U8 = mybir.dt.uint8
AF = mybir.ActivationFunctionType
ALU = mybir.AluOpType
AX = mybir.AxisListType


def _register(name, spec, subdim=False):
    if name in _SUB_OPCODE_FOR_NAME:
        return next(o for o in OPS if o.name == name)
    opcode = 1 + len(OPS)
    shas = {}
    for ver in ("v3", "v4"):
        try:
            u = lower(spec, ver=ver)
            shas[ver] = DveOpSpec(name=name, opcode=opcode, uops=u,
                                  rd1_en=has_src1(spec)).sha(ver)
        except Exception:
            pass
    op = DveOp(name, spec, subdim=subdim, uops_sha=shas)
    OPS.append(op)
    _SUB_OPCODE_FOR_NAME[name] = opcode
    CUSTOM_DVE_SPECS[name] = spec
    return op


def _acc_ref(body_fn, acc=np.add):
    red = {np.add: lambda b: b.sum(-1, keepdims=True),
           np.maximum: lambda b: b.max(-1, keepdims=True)}[acc]

    def _r(in0, in1, s0, s1, imm2):
        b = body_fn(in0, in1, s0, s1, imm2).astype(np.float32)
        b2 = b.reshape(b.shape[0], -1)
        return b, acc(np.asarray(s1, np.float32), red(b2))
    return _r


RELU2B = _register("ENTMAX_RELU2B", Spec(
    body=sq(relu(Src0 + C0)), accum=_add, accum_init=C1,
    reference=_acc_ref(lambda in0, in1, s0, s1, imm2:
                       np.maximum(in0.astype(np.float32) + s0, 0) ** 2),
))
_r3 = relu(Src0 + C0)
RELU3B = _register("ENTMAX_RELU3B", Spec(
    body=sq(_r3) * _r3, accum=_add, accum_init=C1,
    reference=_acc_ref(lambda in0, in1, s0, s1, imm2:
                       np.maximum(in0.astype(np.float32) + s0, 0) ** 3),
))
TGTPICK = _register("ENTMAX_TGTPICK", Spec(
    body=select(eq(Idx, C0), Src0, Zero), accum=_add, accum_init=C1,
    reference=_acc_ref(lambda in0, in1, s0, s1, imm2: np.where(
        np.broadcast_to(np.arange(in0.shape[-1], dtype=np.float32),
                        in0.shape) == s0, in0, 0.0)),
))
CONVMAX = _register("ENTMAX_CONVMAX", Spec(
    body=Src0 + Zero, accum=maxx, accum_init=C1,
    reference=_acc_ref(lambda in0, in1, s0, s1, imm2:
                       in0.astype(np.float32), acc=np.maximum),
))

_NC_CACHE = {}


def _dump_view(dmp, total_cols):
    """AP writing `total_cols` elements cyclically over a rotating dump tile."""
    reps = total_cols // DUMP_COLS
    assert reps * DUMP_COLS == total_cols
    dump = dmp.tile([P, DUMP_COLS], F32, tag="dump")
    return bass.AP(tensor=dump.tensor, offset=dump.offset,
                   ap=[dump.ap[0], [0, reps], dump.ap[1]])


def _seg_view(xh, c0, cols, inner):
    """3-D view of xh[:, c0:c0+cols] shaped [P, cols//inner, inner]."""
    v = xh[:, c0:c0 + cols]
    return v.rearrange("p (a b) -> p a b", a=cols // inner)


def _build():
    if "nc" in _NC_CACHE:
        return _NC_CACHE["nc"]
    nc = bacc.Bacc("TRN2", target_bir_lowering=False, debug=False,
                   num_devices=N_CORES)
    x_d = nc.dram_tensor("x", [ROWS_PER_CORE, V_DIM], F32,
                         kind="ExternalInput").ap()
    tgt_d = nc.dram_tensor("tgt", [ROWS_PER_CORE, 1], mybir.dt.uint16,
                           kind="ExternalInput").ap()
    pmod_d = nc.dram_tensor("pmod", [P, 1], F32, kind="ExternalInput").ap()
    out_d = nc.dram_tensor("out", [1, 1], F32, kind="ExternalOutput").ap()

    n_stage = V_DIM // STAGE_CH
    n_act = V_DIM // ACT_CH
    n_dve = V_DIM // DVE_CH
    spl_dve_cols = SPL_DVE_CH * DVE_CH
    spl_act_cols = V_DIM - spl_dve_cols
    n_splact = spl_act_cols // SPL_ACT_CH
    assert n_splact * SPL_ACT_CH == spl_act_cols

    with tile.TileContext(nc) as tc, ExitStack() as ctx:
        hold = ctx.enter_context(tc.tile_pool(name="hold", bufs=1))
        xpool = ctx.enter_context(tc.tile_pool(name="xpool", bufs=2))
        stg = ctx.enter_context(tc.tile_pool(name="stg", bufs=4))
        rlp = ctx.enter_context(tc.tile_pool(name="rlp", bufs=2))
        small = ctx.enter_context(tc.tile_pool(name="small", bufs=3))
        psum = ctx.enter_context(tc.tile_pool(name="psum", bufs=1, space="PSUM"))
        dmp = ctx.enter_context(tc.tile_pool(name="dmp", bufs=5))

        ones = hold.tile([P, 1], F32)
        nc.vector.memset(ones, 1.0)
        pmod = hold.tile([P, 1], F32)
        nc.sync.dma_start(out=pmod, in_=pmod_d)
        tv = hold.tile([P, GROUPS], F32)
        Wv = hold.tile([P, GROUPS], F32)
        Av = hold.tile([P, GROUPS], F32)
        xtv = hold.tile([P, GROUPS], F32)

        def bracket_update(st, rnd, v_cur, t_new):
            up = small.tile([P, 1], U8, tag=f"up{rnd}")
            nc.vector.tensor_scalar(out=up, in0=v_cur, scalar1=4.0,
                                    scalar2=None, op0=ALU.is_ge)
            lo2 = small.tile([P, 1], F32, tag=f"lo{rnd}")
            hi2 = small.tile([P, 1], F32, tag=f"hi{rnd}")
            nc.vector.select(lo2, up, st["t"], st["lo"])
            nc.vector.select(hi2, up, st["hi"], st["t"])
            mid = small.tile([P, 1], F32, tag=f"md{rnd}")
            nc.vector.tensor_tensor(out=mid, in0=lo2, in1=hi2, op=ALU.add)
            nc.vector.tensor_scalar(out=mid, in0=mid, scalar1=0.5,
                                    scalar2=None, op0=ALU.mult)
            ingt = small.tile([P, 1], U8, tag=f"ig{rnd}")
            inlt = small.tile([P, 1], U8, tag=f"il{rnd}")
            nc.vector.tensor_tensor(out=ingt, in0=t_new, in1=lo2, op=ALU.is_ge)
            nc.vector.tensor_tensor(out=inlt, in0=t_new, in1=hi2, op=ALU.is_le)
            tsel = small.tile([P, 1], F32, tag=f"ts{rnd}")
            nc.vector.select(tsel, ingt, t_new, mid)
            t_next = small.tile([P, 1], F32, tag=f"tx{rnd}")
            nc.vector.select(t_next, inlt, tsel, mid)
            nbias = small.tile([P, 1], F32, tag=f"nb{rnd}")
            nc.vector.tensor_scalar(out=nbias, in0=t_next, scalar1=-1.0,
                                    scalar2=None, op0=ALU.mult)
            st["v_prev"], st["t_prev"] = v_cur, st["t"]
            st["t"], st["lo"], st["hi"], st["nb"] = t_next, lo2, hi2, nbias

        def p_load(g):
            rs = slice(g * P, (g + 1) * P)
            st = {}
            xh = xpool.tile([P, V_DIM], F16, tag="xh")
            mx_slots = small.tile([P, n_stage], F32, tag="mxs")
            for c in range(n_stage):
                stt = stg.tile([P, STAGE_CH], F32, tag="st")
                nc.sync.dma_start(out=stt,
                                  in_=x_d[rs, c * STAGE_CH:(c + 1) * STAGE_CH])
                nc.vector._custom_dve(
                    CONVMAX, out=xh[:, c * STAGE_CH:(c + 1) * STAGE_CH],
                    in0=stt, s0=0.0, s1=-1e30,
                    accum_out=mx_slots[:, c:c + 1])
            tgtu = small.tile([P, 1], mybir.dt.uint16, tag="tgtu")
            nc.sync.dma_start(out=tgtu, in_=tgt_d[rs, :])
            rowmax = small.tile([P, 1], F32, tag="rowmax")
            nc.vector.tensor_reduce(rowmax, mx_slots, axis=AX.X, op=ALU.max)
            lo = small.tile([P, 1], F32, tag="lo_i")
            hi = small.tile([P, 1], F32, tag="hi_i")
            t0 = small.tile([P, 1], F32, tag="t_i")
            nc.vector.tensor_scalar(out=lo, in0=rowmax, scalar1=-2.0,
                                    scalar2=None, op0=ALU.add)
            nc.vector.tensor_scalar(out=hi, in0=rowmax, scalar1=-HI_OFF,
                                    scalar2=None, op0=ALU.add)
            nc.vector.tensor_scalar(out=t0, in0=rowmax, scalar1=-1.0,
                                    scalar2=None, op0=ALU.add)
            nb0 = small.tile([P, 1], F32, tag="nb_i")
            nc.vector.tensor_scalar(out=nb0, in0=rowmax, scalar1=-1.0,
                                    scalar2=1.0, op0=ALU.mult, op1=ALU.add)
            # x[row, tgt] via a dense Idx-select pass (DVE); the GPSIMD
            # indirect_copy gather is faster but crashes the device at
            # >=16000-col width, so stay with the safe dense pick.
            tgtf = small.tile([P, 1], F32, tag="tgtf")
            nc.vector.tensor_copy(tgtf, tgtu)
            p_slots = small.tile([P, n_dve], F32, tag="pfs")
            for c in range(n_dve):
                tadj = small.tile([P, 1], F32, tag=f"ta{c}")
                nc.vector.tensor_scalar(out=tadj, in0=tgtf,
                                        scalar1=-float(c * DVE_CH),
                                        scalar2=None, op0=ALU.add)
                nc.vector._custom_dve(TGTPICK, out=_dump_view(dmp, DVE_CH),
                                      in0=_seg_view(xh, c * DVE_CH, DVE_CH,
                                                    DUMP_COLS),
                                      s0=tadj, s1=0.0,
                                      accum_out=p_slots[:, c:c + 1])
            xt = small.tile([P, 1], F32, tag="xt")
            nc.vector.reduce_sum(xt, p_slots, axis=AX.X)
            nc.vector.tensor_copy(xtv[:, g:g + 1], xt)
            st.update(xh=xh, lo=lo, hi=hi, t=t0, nb=nb0)
            return st

        def p_act_round(g, st, rnd):
            xh, nbias = st["xh"], st["nb"]
            s1_slots = small.tile([P, n_act], F32, tag=f"s1s{rnd}")
            v_slots = small.tile([P, n_act], F32, tag=f"vs{rnd}")
            for c in range(n_act):
                rl = rlp.tile([P, ACT_CH], F32, tag="rl")
                nc.scalar.activation(rl, xh[:, c * ACT_CH:(c + 1) * ACT_CH],
                                     AF.Relu, bias=nbias, scale=1.0,
                                     accum_out=s1_slots[:, c:c + 1])
                nc.scalar.activation(
                    _dump_view(dmp, ACT_CH),
                    rl.rearrange("p (a b) -> p a b", a=ACT_CH // DUMP_COLS),
                    AF.Square, bias=0.0, scale=1.0,
                    accum_out=v_slots[:, c:c + 1])
            v_cur = small.tile([P, 1], F32, tag=f"v{rnd}")
            s1 = small.tile([P, 1], F32, tag=f"s1{rnd}")
            nc.vector.reduce_sum(s1, s1_slots, axis=AX.X)
            nc.vector.reduce_sum(v_cur, v_slots, axis=AX.X)
            denom = small.tile([P, 1], F32, tag=f"dn{rnd}")
            nc.vector.tensor_scalar(out=denom, in0=s1, scalar1=2.0,
                                    scalar2=1e-6, op0=ALU.mult, op1=ALU.max)
            rden = small.tile([P, 1], F32, tag=f"rd{rnd}")
            nc.vector.reciprocal(rden, denom)
            num = small.tile([P, 1], F32, tag=f"nm{rnd}")
            nc.vector.tensor_scalar(out=num, in0=v_cur, scalar1=-4.0,
                                    scalar2=None, op0=ALU.add)
            stp = small.tile([P, 1], F32, tag=f"sp{rnd}")
            nc.vector.tensor_tensor(out=stp, in0=num, in1=rden, op=ALU.mult)
            t_new = small.tile([P, 1], F32, tag=f"tn{rnd}")
            nc.vector.tensor_tensor(out=t_new, in0=st["t"], in1=stp,
                                    op=ALU.add)
            bracket_update(st, rnd, v_cur, t_new)

        def p_split_round(g, st, rnd, update):
            """V eval split across DVE (head cols) + ACT (tail cols)."""
            xh, nbias = st["xh"], st["nb"]
            vd_slots = small.tile([P, SPL_DVE_CH], F32, tag=f"vds{rnd}")
            for c in range(SPL_DVE_CH):
                nc.vector._custom_dve(
                    RELU2B, out=_dump_view(dmp, DVE_CH),
                    in0=_seg_view(xh, c * DVE_CH, DVE_CH, DUMP_COLS),
                    s0=nbias, s1=0.0, accum_out=vd_slots[:, c:c + 1])
            va_slots = small.tile([P, n_splact], F32, tag=f"vas{rnd}")
            for c in range(n_splact):
                c0 = spl_dve_cols + c * SPL_ACT_CH
                rl = rlp.tile([P, SPL_ACT_CH], F32, tag="rl")
                nc.scalar.activation(rl, xh[:, c0:c0 + SPL_ACT_CH],
                                     AF.Relu, bias=nbias, scale=1.0)
                nc.scalar.activation(
                    _dump_view(dmp, SPL_ACT_CH),
                    rl.rearrange("p (a b) -> p a b",
                                 a=SPL_ACT_CH // DUMP_COLS),
                    AF.Square, bias=0.0, scale=1.0,
                    accum_out=va_slots[:, c:c + 1])
            vh = small.tile([P, 1], F32, tag=f"vh{rnd}")
            va = small.tile([P, 1], F32, tag=f"va{rnd}")
            nc.vector.reduce_sum(vh, vd_slots, axis=AX.X)
            nc.vector.reduce_sum(va, va_slots, axis=AX.X)
            v_cur = small.tile([P, 1], F32, tag=f"v{rnd}")
            nc.vector.tensor_tensor(out=v_cur, in0=vh, in1=va, op=ALU.add)
            if not update:
                st["v_cur"] = v_cur
                return
            # secant: tN = t - (V-4)*(t - t_prev)/min(V - V_prev, -eps)
            dv = small.tile([P, 1], F32, tag=f"dv{rnd}")
            nc.vector.tensor_tensor(out=dv, in0=v_cur, in1=st["v_prev"],
                                    op=ALU.subtract)
            dvg = small.tile([P, 1], F32, tag=f"dvg{rnd}")
            nc.vector.tensor_scalar(out=dvg, in0=dv, scalar1=-1e-6,
                                    scalar2=None, op0=ALU.min)
            rdv = small.tile([P, 1], F32, tag=f"rdv{rnd}")
            nc.vector.reciprocal(rdv, dvg)
            dt = small.tile([P, 1], F32, tag=f"dt{rnd}")
            nc.vector.tensor_tensor(out=dt, in0=st["t"], in1=st["t_prev"],
                                    op=ALU.subtract)
            num = small.tile([P, 1], F32, tag=f"nm{rnd}")
            nc.vector.tensor_scalar(out=num, in0=v_cur, scalar1=-4.0,
                                    scalar2=None, op0=ALU.add)
            sl = small.tile([P, 1], F32, tag=f"sl{rnd}")
            nc.vector.tensor_tensor(out=sl, in0=dt, in1=rdv, op=ALU.mult)
            stp = small.tile([P, 1], F32, tag=f"st{rnd}")
            nc.vector.tensor_tensor(out=stp, in0=num, in1=sl, op=ALU.mult)
            t_new = small.tile([P, 1], F32, tag=f"tn{rnd}")
            nc.vector.tensor_tensor(out=t_new, in0=st["t"], in1=stp,
                                    op=ALU.subtract)
            bracket_update(st, rnd, v_cur, t_new)

        def p_w_store(g, st):
            W = st["v_cur"]  # R3's V at st["t"]
            nc.vector.tensor_copy(Wv[:, g:g + 1], W)
            nc.vector.tensor_copy(tv[:, g:g + 1], st["t"])

        def p_a_pass(g, st):
            xh, nbias = st["xh"], st["nb"]
            a_slots = small.tile([P, n_dve], F32, tag="afs")
            for c in range(n_dve):
                nc.vector._custom_dve(RELU3B, out=_dump_view(dmp, DVE_CH),
                                      in0=_seg_view(xh, c * DVE_CH, DVE_CH,
                                                    DUMP_COLS),
                                      s0=nbias, s1=0.0,
                                      accum_out=a_slots[:, c:c + 1])
            A = small.tile([P, 1], F32, tag="Af")
            nc.vector.reduce_sum(A, a_slots, axis=AX.X)
            nc.vector.tensor_copy(Av[:, g:g + 1], A)

        # pipelined emission: next group's load after this group's R0
        states = {0: p_load(0)}
        for g in range(GROUPS):
            p_act_round(g, states[g], 0)
            if g + 1 < GROUPS:
                states[g + 1] = p_load(g + 1)
            p_split_round(g, states[g], 1, update=True)
            p_split_round(g, states[g], 2, update=True)
            p_split_round(g, states[g], 3, update=False)
            p_w_store(g, states[g])
            p_a_pass(g, states[g])

        # ---- loss assembly for all groups at once ([P, GROUPS]) ----
        Wg = hold.tile([P, GROUPS], F32)
        nc.vector.tensor_scalar(out=Wg, in0=Wv, scalar1=1e-20, scalar2=None,
                                op0=ALU.max)
        y0 = hold.tile([P, GROUPS], F32)
        nc.scalar.activation(y0, Wg, AF.Sqrt, bias=0.0, scale=1.0)
        ry = hold.tile([P, GROUPS], F32)
        nc.vector.reciprocal(ry, y0)
        wry = hold.tile([P, GROUPS], F32)
        nc.vector.tensor_tensor(out=wry, in0=Wg, in1=ry, op=ALU.mult)
        y1 = hold.tile([P, GROUPS], F32)
        nc.vector.tensor_tensor(out=y1, in0=wry, in1=y0, op=ALU.add)
        nc.vector.tensor_scalar(out=y1, in0=y1, scalar1=0.5, scalar2=None,
                                op0=ALU.mult)
        w15 = hold.tile([P, GROUPS], F32)
        nc.vector.tensor_tensor(out=w15, in0=Wg, in1=y1, op=ALU.mult)
        r15 = hold.tile([P, GROUPS], F32)
        nc.vector.reciprocal(r15, w15)
        rW = hold.tile([P, GROUPS], F32)
        nc.vector.reciprocal(rW, Wg)
        sp15 = hold.tile([P, GROUPS], F32)
        nc.vector.tensor_tensor(out=sp15, in0=Av, in1=r15, op=ALU.mult)
        aw = hold.tile([P, GROUPS], F32)
        nc.vector.tensor_tensor(out=aw, in0=Av, in1=rW, op=ALU.mult)
        l1 = hold.tile([P, GROUPS], F32)
        nc.vector.tensor_scalar(out=l1, in0=sp15, scalar1=-4.0 / 3.0,
                                scalar2=4.0 / 3.0, op0=ALU.mult, op1=ALU.add)
        l2 = hold.tile([P, GROUPS], F32)
        nc.vector.tensor_tensor(out=l2, in0=l1, in1=aw, op=ALU.add)
        l3 = hold.tile([P, GROUPS], F32)
        nc.vector.tensor_tensor(out=l3, in0=l2, in1=tv, op=ALU.add)
        lossm = hold.tile([P, GROUPS], F32)
        nc.vector.tensor_tensor(out=lossm, in0=l3, in1=xtv, op=ALU.subtract)
        loss_acc = hold.tile([P, 1], F32)
        nc.vector.reduce_sum(loss_acc, lossm, axis=AX.X)

        acc_ps = psum.tile([1, 1], F32, tag="acc_ps")
        nc.tensor.matmul(acc_ps, lhsT=loss_acc, rhs=ones, start=True,
                         stop=True)
        acc_sb = small.tile([1, 1], F32, tag="acc_sb")
        nc.scalar.activation(acc_sb, acc_ps, AF.Copy, bias=0.0, scale=1.0)
        nc.sync.dma_start(out=out_d, in_=acc_sb)


    nc.compile()
    _NC_CACHE["nc"] = nc
    return nc


def _in_maps(x, tgt):
    pmod = (np.arange(P) % 16).astype(np.float32).reshape(P, 1)
    maps = []
    for i in range(N_CORES):
        sl = slice(i * ROWS_PER_CORE, (i + 1) * ROWS_PER_CORE)
        maps.append({
            "x": x[sl],
            "tgt": tgt[sl].astype(np.uint16).reshape(ROWS_PER_CORE, 1),
            "pmod": pmod,
        })
    return maps


def kernel(input, target):
    x = np.ascontiguousarray(np.asarray(input, dtype=np.float32))
    tgt = np.asarray(target).astype(np.int64)
    assert x.shape == (N_ROWS, V_DIM)
    nc = _build()
    r = run_bass_kernel_spmd(nc, _in_maps(x, tgt), core_ids=list(range(N_CORES)))
    total = np.float64(0.0)
    for i in range(N_CORES):
        total += np.float64(r.results[i]["out"][0, 0])
    return np.asarray(np.float32(total / N_ROWS))


if __name__ == "__main__":
    rng = np.random.default_rng(0)
    x = rng.standard_normal((N_ROWS, V_DIM)).astype(np.float32)
    t = rng.integers(0, V_DIM, (N_ROWS,)).astype(np.int64)
    print("loss:", kernel(input=x, target=t))

